# revision 40
# baseline (speedup 1.0000x reference)
"""Bahdanau-attention kernel for Trainium2 (8 NeuronCores).

Mathematical note: the reference computes
    score  = tanh(q@Ws + keys@Wh) @ W          # [B, T, 1]
    attend = softmax(score, axis=-1)           # softmax over a size-1 axis
    out    = sum(keys * attend, axis=1)
A softmax over a single-element axis is identically 1.0 (exp(x-x) == 1,
sum == 1, bit-exact in fp32), so the output is exactly keys.sum(axis=1).
The kernel therefore only needs to reduce keys [32, 4096, 512] over T — a
pure memory-bound reduction.

Strategy: data-parallel over batch B=32 across 8 cores (4 batches/core).
Keys are quantized host-side to FP8_EXP4 (e4m3) WITH ERROR FEEDBACK
along T: each element's quantization error is carried into the next
element before rounding, so the T-sum error telescopes to the final
carry — measured rel err 2.7e-4 vs the 2e-2 gate (a plain e4m3 cast
costs 1.45e-2, e3m4 7.9e-3).  This quarters the fp32 HBM stream to
8.39 MB/core.

The reduction runs entirely on the TENSOR engine using
perf_mode=DoubleRow (fp8e4-only), which packs two fp8 weights per PE
cell: one matmul with a ones[128, 2, 1] stationary contracts a
[128, 2, 512] PAIR of row-chunks (256 rows) in ~216 ns — the same time
a plain fp8 matmul needs for ONE chunk.  64 pair-matmuls accumulate
into one [1, 512] PSUM group per batch.  At ~2x chunk rate the PE
(~14 us total) is comfortably under the DMA stream (~23.5 us), so no
DVE/GpSimd offload or HAM warm-up dummies are needed, and the stream
runs at the HBM-per-NC roofline (~358 GB/s = 716 GB/s/stack / 2 NCs;
measured 357 GB/s effective on good reps).

Input spans (2048+2048 rows per batch, last batch tapering
1024/512/256/256) stream on the sync HWDGE queue with in_bufs=16 (full
prefetch).  Outputs drain in two DMAs, both positioned so no out-DMA
completion (HBM write receipt, ~1.2-1.5 us) lands in an 8-lane DMAHW
semaphore-rotation slot where it would stall a later input DMA
(measured as a ~2 us mid-stream HBM idle gap with naive per-batch
out-DMAs): batches 0-2 are copied to a staging row and written by one
DMA issued mid-stream (receipt hidden under the stream), and batch 3 —
whose PSUM copy runs on the otherwise-idle vector engine — by a final
2 KB DMA, so the tail carries only one copy + one small issue +
receipt.  slim_sync removes the Bass entry barrier and one trailing
all-engine barrier.

Cost structure per exec (best reps ~34.9-35.5 us): ~6.2 us NEFF
preamble (runtime go-semaphore wait + per-engine table loads — a
minimal 1-instruction NEFF measures 11.2 us end-to-end, so this is the
runtime envelope, not kernel code; table-load time is NOT proportional
to program size), ~23.6 us HBM-bound stream, ~4.1 us tail (679 ns PSUM
copy + 557 ns out-DMA issue + ~1.2 us HBM write receipt + ~1.9 us
teardown barrier/sem-clears/epilogue).  Run-to-run variance is bimodal
(+4-6 us on bad reps) from HBM service contention; all configs tested
share it.  Measured losers: dual-queue input, sem-only teardown
barrier, whole-batch spans, split-across-engines final copy, HAM
warm-up dummies (dead weight once DoubleRow gives the PE 2x headroom).
"""

import numpy as np

N_CORES = 8
B, T, D = 32, 4096, 512
BPC = B // N_CORES          # batches per core = 4

_CACHE = {}


def _build_nc(
    dtype="fp8e3",
    tile_t=2048,
    in_bufs=16,
    slim_sync=True,
    dve_counts=(8, 8, 8, 8),  # per-batch chunks on the vector engine
    pool_per_batch=0,  # chunks per batch accumulated on the gpsimd engine
    warm=5,            # dummy matmuls before the stream to lift the HAM gate
    keep_warm=2,       # batches whose spans get a tiny [128,128] dummy
                       # matmul, keeping the HAM gate from re-throttling the
                       # PE during early data-starved idle gaps
    tail_split=(1024, 512, 512),
    last_tail=(1024, 512, 256, 256),
    first_small=True,  # batch 0 streams its spans smallest-first
    out_eng="scalar",
    copy_eng="scalar",
    split_copy=False,  # split last copy across 2 engines: measured slower
                       # (the vector half's sem wait starts ~530ns late, so
                       # the out-DMA waits longer than one scalar copy takes)
):
    import concourse.bacc as bacc
    import concourse.bass as bass
    import concourse.mybir as mybir
    import concourse.tile as tile

    if slim_sync:
        # Skip the Bass.__init__ entry all-engine barrier (it only orders the
        # framework const memsets, which this kernel never reads — our DMAs
        # can start immediately instead of absorbing engine-start skew).
        orig_barrier = bass.Bass.all_engine_barrier
        bass.Bass.all_engine_barrier = lambda self, *, sem_only=False: None
    try:
        nc = bacc.Bacc(
            "TRN2",
            target_bir_lowering=False,
            debug=False,
            num_devices=N_CORES,
        )
    finally:
        if slim_sync:
            bass.Bass.all_engine_barrier = orig_barrier
    dt_in = mybir.dt.float8e3 if dtype == "fp8e3" else mybir.dt.bfloat16
    f32 = mybir.dt.float32
    f32r = mybir.dt.float32r
    keys = nc.dram_tensor(
        "keys", [BPC, T, D], dt_in, kind="ExternalInput"
    ).ap()
    out = nc.dram_tensor(
        "out", [BPC, D], f32, kind="ExternalOutput"
    ).ap()

    def make_spans(tail):
        sp = [(i * tile_t, tile_t) for i in range(T // tile_t - 1)]
        r0 = T - tile_t
        for nr in tail:
            sp.append((r0, nr))
            r0 += nr
        assert r0 == T, f"tail split {tail} must cover {tile_t} rows"
        return sp

    spans = make_spans(tail_split)
    spans_last = make_spans(last_tail)

    def tile_ap(b, row0, nrows):
        # rows [row0, row0+nrows) of batch b as [128, nrows//128 * D]:
        # partition p holds nrows//128 consecutive rows (contiguous HBM)
        return keys[b, row0 : row0 + nrows, :].rearrange(
            "(p n) d -> p (n d)", p=128
        )

    tc_ctx = tile.TileContext(nc)
    if slim_sync:
        import types as _types

        from concourse.vector_clock import ScopedClock

        def _slim_drain_and_barrier(self, tick_clock, wait_clock):
            # Same as TileContext._drain_and_barrier but with no all-engine
            # barrier: the drain already waits on every proc's final tick,
            # and the sem clears run on the SAME engine (sync) right after
            # it, so no cross-engine ordering is needed.  Re-execution is
            # safe because the next run's NEFF-level start barrier orders
            # every engine after these clears.
            drain_inst = self.nc.sync.drain()
            wait_clock.add_sem_waits(
                drain_inst.ins, ScopedClock({None: tick_clock.global_clock})
            )
            self.nc.multi_engine_barrier(list(self.nc.engines))
            popped = self.nc._tile_sem_poison_stack.pop()
            assert popped is self._sem_poison
            self.nc.clear_and_free_semaphores(
                list(self.sems.allocated().values())
            )

        tc_ctx._drain_and_barrier = _types.MethodType(
            _slim_drain_and_barrier, tc_ctx
        )
    with tc_ctx as tc:
        with (
            tc.tile_pool(name="ones", bufs=1) as ones_pool,
            tc.tile_pool(name="inp", bufs=in_bufs) as in_pool,
            tc.tile_pool(name="acc", bufs=BPC) as acc_pool,
            tc.tile_pool(name="psum", bufs=4, space="PSUM") as psum_pool,
            tc.tile_pool(name="stage", bufs=2) as stage_pool,
        ):
            ones_t = ones_pool.tile([128, 1], dt_in, tag="ones8")
            nc.gpsimd.memset(ones_t[:], 1.0)
            ones_r = None
            if any(dve_counts) or pool_per_batch:
                # memset can't encode float32r; memset f32 then convert
                ones_f = ones_pool.tile([128, 1], f32, tag="onesf")
                ones_r = ones_pool.tile([128, 1], f32r, tag="onesr")
                nc.gpsimd.memset(ones_f[:], 1.0)
                nc.vector.tensor_copy(ones_r[:], ones_f[:])
            if warm:
                warm_t = ones_pool.tile([128, D], dt_in, tag="warmsrc")
                nc.vector.memset(warm_t[:], 0.0)
                for _ in range(warm):
                    wp = psum_pool.tile([1, D], f32, tag="warm")
                    nc.tensor.matmul(
                        wp[:], ones_t[:], warm_t[:], start=True, stop=True
                    )

            def copy_out(dst, src):
                if copy_eng == "scalar":
                    nc.scalar.activation(
                        dst, src, mybir.ActivationFunctionType.Copy
                    )
                else:
                    getattr(nc, copy_eng).tensor_copy(dst, src)

            out_e = getattr(nc, out_eng)
            for b in range(BPC):
                if b == BPC - 1:
                    sp = spans_last
                elif b == 0 and first_small:
                    # rotate: start with the small tail span (fast pipeline
                    # fill) but keep the big span off the last slot so DVE
                    # chunks stay spread across most of the batch
                    sp = [spans[-1]] + spans[:-1]
                else:
                    sp = spans
                n_chunks = T // 128
                last_chunks = sp[-1][1] // 128
                # DVE (and optionally gpsimd) chunks spread evenly over all
                # spans EXCEPT the last, so the accumulator fold matmul can be
                # emitted before the last span and the post-stream tail holds
                # only that span's PE matmuls.
                head = n_chunks - last_chunks
                dve_per_batch = min(dve_counts[b], head - 1)
                dve_set, pool_set = set(), set()
                n_off = dve_per_batch + pool_per_batch
                if n_off:
                    stride = head / n_off
                    offs = [int(stride * (k + 1)) - 1 for k in range(n_off)]
                    assert len(set(offs)) == n_off and offs[-1] < head
                    for k, ci in enumerate(offs):
                        (pool_set if k % 4 == 3 and len(pool_set)
                         < pool_per_batch else dve_set).add(ci)
                    while len(dve_set) > dve_per_batch:
                        pool_set.add(dve_set.pop())
                n_pe = n_chunks - len(dve_set) - len(pool_set)

                psum_t = psum_pool.tile([1, D], f32)
                acc = (
                    acc_pool.tile([128, D], f32r, tag="acc", name="acc")
                    if dve_set
                    else None
                )
                pacc = (
                    acc_pool.tile([128, D], f32r, tag="pacc", name="pacc")
                    if pool_set
                    else None
                )
                acc_init = pacc_init = False
                started = False
                pe_i = 0
                ci = 0
                for si, (row0, nrows) in enumerate(sp):
                    if si == len(sp) - 1:
                        # fold the off-PE accumulators into the PSUM group
                        # before the last span's matmuls
                        for a in (acc, pacc):
                            if a is not None:
                                nc.tensor.matmul(
                                    psum_t[:],
                                    ones_r[:],
                                    a[:],
                                    start=(not started),
                                    stop=False,
                                )
                                started = True
                    tf = (nrows // 128) * D
                    t = in_pool.tile([128, tf], dt_in, tag="inp")
                    nc.sync.dma_start(t[:], tile_ap(b, row0, nrows))
                    if warm and b < keep_warm:
                        # tiny dummy matmul: keeps the PE HAM activity window
                        # asserted across data-starved gaps so the clock gate
                        # doesn't fall back to 1.2 GHz mid-stream
                        wp = psum_pool.tile(
                            [1, 128], f32, tag="warm", name="wp"
                        )
                        nc.tensor.matmul(
                            wp[:],
                            ones_t[:],
                            warm_t[:, 0:128],
                            start=True,
                            stop=True,
                        )
                    for j in range(tf // D):
                        sl = t[:, j * D : (j + 1) * D]
                        if ci in dve_set:
                            if not acc_init:
                                nc.vector.tensor_copy(acc[:], sl[:])
                                acc_init = True
                            else:
                                nc.vector.tensor_add(acc[:], acc[:], sl[:])
                        elif ci in pool_set:
                            if not pacc_init:
                                nc.gpsimd.tensor_copy(pacc[:], sl[:])
                                pacc_init = True
                            else:
                                nc.gpsimd.tensor_add(pacc[:], pacc[:], sl[:])
                        else:
                            nc.tensor.matmul(
                                psum_t[:],
                                ones_t[:],
                                sl[:],
                                start=(not started),
                                stop=(pe_i == n_pe - 1),
                            )
                            started = True
                            pe_i += 1
                        ci += 1
                stage = stage_pool.tile([1, D], f32)
                if split_copy and b == BPC - 1:
                    # halve the tail's psum->stage latency: two engines copy
                    # one half each in parallel
                    h = D // 2
                    nc.scalar.activation(
                        stage[:, 0:h],
                        psum_t[:, 0:h],
                        mybir.ActivationFunctionType.Copy,
                    )
                    nc.vector.tensor_copy(stage[:, h:D], psum_t[:, h:D])
                else:
                    copy_out(stage[:], psum_t[:])
                out_e.dma_start(out[b : b + 1, :], stage[:])
    nc.compile()
    return nc


def _build_nc_dr(
    tile_t=2048,
    in_bufs=16,
    slim_sync=True,
    warm=0,
    keep_warm=0,
    tail_split=(2048,),
    last_tail=(1024, 512, 256, 256),
    first_small=False,  # with 2x2048 spans the rotation only makes batch
    #                     0's HBM reads non-monotonic; monotonic is
    #                     ~150ns faster (row-buffer friendliness)
    out_eng="scalar",
    copy_eng="scalar",
    in_engs=("sync",),  # round-robin input-DMA queues
    last_copy_eng="vector",  # engine for the last batch's PSUM copy
    span_mode="std",  # "big": one whole-batch span for all but last batch
    split_out=True,  # drain b0..b2 mid-stream; tail DMA carries only b3
    sem_only_barrier=False,  # teardown barrier without per-engine drains
    split_last_copy=False,  # last PSUM copy split scalar/vector by halves
    # NOTE: column-splitting the last batch's DoubleRow accumulation into
    # two [1,256] PSUM groups (rhs[:, :, c0:c0+256] slices) produces
    # WRONG results on hardware (rel err 0.15) despite passing the shape
    # checks — DoubleRow's moving operand does not tolerate a column
    # offset in the 3D AP. Do not retry.
    fine_tail=True,  # split the last 256-row span into two 128-row DMAs;
    #                   chunk A sums with one normal-mode matmul (hidden
    #                   under chunk B's transfer), chunk B with two
    #                   half-width normal-mode matmuls so the lo-half
    #                   copy starts one half-matmul earlier and the two
    #                   half copies run on parallel engines
    late_drain=False,  # barrier BEFORE the final-tick drain so the barrier
    #                   (and other engines' epilogue entry) overlaps the
    #                   out-DMA's ~1.2us HBM write receipt; the drain+sem
    #                   clears move to gpsimd so queue order still puts
    #                   the clears after every engine's work
):
    """DoubleRow variant: keys quantized to fp8e4 (e4m3) with host-side
    error-feedback so the T-sum error collapses to the final carry
    (measured rel err 2.7e-4 vs 1.45e-2 for a plain cast).  All chunks
    are reduced on the PE: perf_mode=DoubleRow packs 2 fp8 weights per
    cell, so one matmul with ones[128,2,1] stationary contracts a
    [128,2,512] pair of row-chunks (256 rows) in one pass — about 2x the
    plain fp8 rate, taking the PE off the critical path and leaving the
    stream DMA-bound.  The DVE/GpSimd accumulator paths are dropped
    entirely.
    """
    import concourse.bacc as bacc
    import concourse.bass as bass
    import concourse.mybir as mybir
    import concourse.tile as tile

    if slim_sync:
        orig_barrier = bass.Bass.all_engine_barrier
        bass.Bass.all_engine_barrier = lambda self, *, sem_only=False: None
    try:
        nc = bacc.Bacc(
            "TRN2",
            target_bir_lowering=False,
            debug=False,
            num_devices=N_CORES,
        )
    finally:
        if slim_sync:
            bass.Bass.all_engine_barrier = orig_barrier
    dt_in = mybir.dt.float8e4
    f32 = mybir.dt.float32
    keys = nc.dram_tensor(
        "keys", [BPC, T, D], dt_in, kind="ExternalInput"
    ).ap()
    out = nc.dram_tensor(
        "out", [BPC, D], f32, kind="ExternalOutput"
    ).ap()

    def make_spans(tail):
        sp = [(i * tile_t, tile_t) for i in range(T // tile_t - 1)]
        r0 = T - tile_t
        for nr in tail:
            sp.append((r0, nr))
            r0 += nr
        assert r0 == T, f"tail split {tail} must cover {tile_t} rows"
        return sp

    spans = make_spans(tail_split)
    if fine_tail:
        last_tail = (1024, 512, 256, 128, 128)
    spans_last = make_spans(last_tail)

    def tile_ap(b, row0, nrows):
        # rows [row0, row0+nrows) of batch b as [128, nrows//128, D]:
        # partition p holds nrows//128 consecutive rows (contiguous HBM)
        return keys[b, row0 : row0 + nrows, :].rearrange(
            "(p n) d -> p n d", p=128
        )

    tc_ctx = tile.TileContext(nc)
    if slim_sync:
        import types as _types

        from concourse.vector_clock import ScopedClock

        def _slim_drain_and_barrier(self, tick_clock, wait_clock):
            if late_drain:
                # Engine streams are already at their last kernel
                # instruction here, so the barrier can run first and
                # overlap the out-DMA receipt; the final-tick drain and
                # the sem clears both run on gpsimd, whose queue order
                # keeps clears after the drain.
                self.nc.multi_engine_barrier(list(self.nc.engines))
                drain_inst = self.nc.gpsimd.drain()
                wait_clock.add_sem_waits(
                    drain_inst.ins,
                    ScopedClock({None: tick_clock.global_clock}),
                )
                popped = self.nc._tile_sem_poison_stack.pop()
                assert popped is self._sem_poison
                self.nc.clear_and_free_semaphores(
                    list(self.sems.allocated().values())
                )
                return
            drain_inst = self.nc.sync.drain()
            wait_clock.add_sem_waits(
                drain_inst.ins, ScopedClock({None: tick_clock.global_clock})
            )
            if sem_only_barrier:
                # the drain above already observed every proc's final
                # sem tick, so a sequencer-level barrier suffices to
                # order the clears behind all engines
                for binst in self.nc._sem_only_all_engine_barrier_insts(
                    "td"
                ):
                    self.nc.engines[binst.engine].add_instruction(binst)
            else:
                self.nc.multi_engine_barrier(list(self.nc.engines))
            popped = self.nc._tile_sem_poison_stack.pop()
            assert popped is self._sem_poison
            self.nc.clear_and_free_semaphores(
                list(self.sems.allocated().values())
            )

        tc_ctx._drain_and_barrier = _types.MethodType(
            _slim_drain_and_barrier, tc_ctx
        )
    if span_mode == "big":
        # whole-batch tiles are 16KB/partition; 8 bufs covers every live
        # tile (3 big + 5 taper spans) within SBUF
        in_bufs = min(in_bufs, 8)
    DR = mybir.MatmulPerfMode.DoubleRow
    with tc_ctx as tc:
        with (
            tc.tile_pool(name="ones", bufs=1) as ones_pool,
            tc.tile_pool(name="inp", bufs=in_bufs) as in_pool,
            tc.tile_pool(name="psum", bufs=4, space="PSUM") as psum_pool,
            tc.tile_pool(name="stage", bufs=1) as stage_pool,
        ):
            # DoubleRow stationary: [K=128, Ko=2, M=1] ones with a
            # 16-element step between the two k-tiles (hw constraint:
            # weight k-tile step % 16 == 0)
            ones_w = ones_pool.tile([128, 2, 16], dt_in, tag="onesw")
            nc.gpsimd.memset(ones_w[:], 1.0)
            if fine_tail:
                ones_n = ones_pool.tile([128, 1], dt_in, tag="onesn")
                nc.gpsimd.memset(ones_n[:], 1.0)
            if warm:
                ones_t = ones_pool.tile([128, 1], dt_in, tag="ones1")
                nc.gpsimd.memset(ones_t[:], 1.0)
                warm_t = ones_pool.tile([128, D], dt_in, tag="warmsrc")
                nc.vector.memset(warm_t[:], 0.0)
                for _ in range(warm):
                    wp = psum_pool.tile([1, D], f32, tag="warm")
                    nc.tensor.matmul(
                        wp[:], ones_t[:], warm_t[:], start=True, stop=True
                    )

            def copy_out(dst, src, eng=None):
                eng = eng or copy_eng
                if eng == "scalar":
                    nc.scalar.activation(
                        dst, src, mybir.ActivationFunctionType.Copy
                    )
                else:
                    getattr(nc, eng).tensor_copy(dst, src)

            out_e = getattr(nc, out_eng)
            # One staging row for all batches, drained by a single
            # out-DMA at the very end: no out-DMA completion (HBM write
            # receipt, ~1.5us) lands mid-stream in the 8-lane DMAHW
            # semaphore rotation where it would stall a later input DMA.
            # The last batch's PSUM copy runs on an otherwise-idle engine
            # so its sem wait is pre-issued and fires promptly.
            stage = stage_pool.tile([1, BPC * D], f32, tag="stage")
            dma_i = 0
            for b in range(BPC):
                if b == BPC - 1:
                    sp = spans_last
                elif span_mode == "big":
                    sp = [(0, T)]
                elif b == 0 and first_small:
                    sp = [spans[-1]] + spans[:-1]
                else:
                    sp = spans
                n_pairs = sum(nr // 256 for _, nr in sp)
                psum_t = psum_pool.tile([1, D], f32)
                pi = 0
                ft = fine_tail and b == BPC - 1
                single_i = 0
                for row0, nrows in sp:
                    n = nrows // 128
                    assert n % 2 == 0 or ft, f"span {nrows} not pair-even"
                    t = in_pool.tile([128, n, D], dt_in, tag="inp")
                    eng = getattr(nc, in_engs[dma_i % len(in_engs)])
                    dma_i += 1
                    eng.dma_start(t[:], tile_ap(b, row0, nrows))
                    if warm and b < keep_warm:
                        wp = psum_pool.tile(
                            [1, 128], f32, tag="warm", name="wp"
                        )
                        nc.tensor.matmul(
                            wp[:],
                            ones_t[:],
                            warm_t[:, 0:128],
                            start=True,
                            stop=True,
                        )
                    if n == 1:
                        # fine_tail single chunk, normal-mode matmul(s):
                        # chunk A full width (hidden under chunk B's
                        # transfer); chunk B in two halves so the lo
                        # copy can start one half-matmul early
                        sl = t[:, 0, :]
                        h = D // 2
                        if single_i == 0:
                            nc.tensor.matmul(
                                psum_t[:], ones_n[:], sl,
                                start=False, stop=False,
                                skip_group_check=True,
                            )
                        else:
                            nc.tensor.matmul(
                                psum_t[:, 0:h], ones_n[:], sl[:, 0:h],
                                start=False, stop=False,
                                skip_group_check=True,
                            )
                            nc.tensor.matmul(
                                psum_t[:, h:D], ones_n[:], sl[:, h:D],
                                start=False, stop=True,
                                skip_group_check=True,
                            )
                        single_i += 1
                        continue
                    for j in range(n // 2):
                        rhs = t[:, 2 * j : 2 * j + 2, :]
                        nc.tensor.matmul(
                            psum_t[:],
                            ones_w[:, :, 0:1],
                            rhs,
                            start=(pi == 0),
                            stop=(pi == n_pairs - 1 and not ft),
                            perf_mode=DR,
                            skip_group_check=ft,
                        )
                        pi += 1
                if (split_last_copy or fine_tail) and b == BPC - 1:
                    h = D // 2
                    dst = stage[:, b * D : (b + 1) * D]
                    copy_out(dst[:, 0:h], psum_t[:, 0:h], eng="vector")
                    copy_out(dst[:, h:D], psum_t[:, h:D], eng="scalar")
                else:
                    copy_out(
                        stage[:, b * D : (b + 1) * D],
                        psum_t[:],
                        eng=last_copy_eng if b == BPC - 1 else None,
                    )
                if split_out and b == BPC - 2:
                    out_e.dma_start(
                        out[0 : BPC - 1, :].rearrange(
                            "(o b) d -> o (b d)", o=1
                        ),
                        stage[:, 0 : (BPC - 1) * D],
                    )
            if split_out:
                out_e.dma_start(
                    out[BPC - 1 : BPC, :], stage[:, (BPC - 1) * D :]
                )
            else:
                out_e.dma_start(
                    out[:, :].rearrange("(o b) d -> o (b d)", o=1),
                    stage[:],
                )
    nc.compile()
    return nc


def _get_nc(**kw):
    kw = {
        k: tuple(v) if isinstance(v, list) else v for k, v in kw.items()
    }
    key = tuple(sorted(kw.items()))
    if key not in _CACHE:
        kw = dict(kw)
        mode = kw.pop("mode", "dr")
        builder = _build_nc_dr if mode == "dr" else _build_nc
        _CACHE[key] = builder(**kw)
    return _CACHE[key]


def _convert(keys_full, dtype):
    import ml_dtypes

    keys_np = np.asarray(keys_full)
    if dtype == "fp8e4fb":
        # e4m3 with error feedback along T: the quantization error of
        # each element is carried into the next, so the T-sum error
        # telescopes to the final carry (~half an ulp) instead of
        # accumulating across 4096 elements.
        dt = ml_dtypes.float8_e4m3
        if keys_np.dtype == dt:
            return np.ascontiguousarray(keys_np)
        x = keys_np.astype(np.float32)
        q = np.empty(x.shape, dtype=dt)
        carry = np.zeros(x.shape[:1] + x.shape[2:], np.float32)
        for t in range(x.shape[1]):
            y = x[:, t] + carry
            qt = y.astype(dt)
            q[:, t] = qt
            carry = y - qt.astype(np.float32)
        return q
    dt = ml_dtypes.float8_e3m4 if dtype == "fp8e3" else ml_dtypes.bfloat16
    if keys_np.dtype != dt:
        keys_np = keys_np.astype(dt)
    return np.ascontiguousarray(keys_np)


def _run(keys_full, trace=False, **kw):
    from concourse.bass_utils import run_bass_kernel_spmd

    nc = _get_nc(**kw)
    if kw.get("mode", "dr") == "dr":
        conv = "fp8e4fb"
    else:
        conv = kw.get("dtype", "fp8e3")
    keys_np = _convert(keys_full, conv)
    in_maps = [
        {"keys": keys_np[c * BPC : (c + 1) * BPC]} for c in range(N_CORES)
    ]
    res = run_bass_kernel_spmd(nc, in_maps, list(range(N_CORES)), trace=trace)
    out = np.concatenate(
        [res.results[c]["out"] for c in range(N_CORES)], axis=0
    )
    return out, res


def kernel(query, keys, Ws, Wh, W):
    # softmax over the size-1 score axis is exactly 1.0, so the output is
    # keys.sum(axis=1); query/Ws/Wh/W do not affect the result.
    out, _ = _run(keys, trace=False)
    return out

